# revision 1
# baseline (speedup 1.0000x reference)
"""Trainium2 Bass kernel for nn_CorrBlockSingleScale (RAFT single-scale
correlation lookup), distributed over 8 NeuronCores.

  fmap1, fmap2: [1, 256, 64, 96] f32;  coords: [1, 2, 64, 96] f32; radius=4
  corr = einsum('bcm,bcn->bmn', f1, f2) / 16        -> [6144, 64, 96]
  out[q, i, j] = bilinear(corr[q], (cx_q + d_i, cy_q + d_j)),  d in -4..4
  output [1, 81, 64, 96] f32.

Structure exploited: the 9x9 sample offsets are integers, so all 81 samples
of a query share one fractional pair (fx, fy) -- the output is a separable
2x2-tap blend of a 10x10 patch of corr[q] anchored at
(floor(cx)-4, floor(cy)-4).

Distribution (no collectives): queries are sorted by floor(cy) on the host;
each core takes 768 contiguous sorted queries and therefore only needs a
narrow y-band (~19 of 64 rows) of the correlation target plane.  Per core:
  1. matmul f1_tile^T @ f2_band with K=256 split into bf16 hi/lo pairs
     (3 accumulating matmuls per K-half: hi*hi, hi*lo, lo*hi -- fp32-class
     accuracy at bf16 PE throughput).  Band columns are host-permuted to
     x-major order so each query's corr band lands transposed in DRAM.
  2. DMA the band to a per-tile DRAM scratch slot per query.
  3. indirect-DMA gather one contiguous window per query (the 10x10 patch
     spans 9*W_ROWS+10 elements in the x-major layout).
  4. blend the patch with host-folded bilinear weights + validity masks on
     the vector engine; DMA out [128, 81] rows.
Host post-pass inverse-permutes and transposes to the reference layout.
"""


import numpy as np

import concourse.bass as bass
import concourse.bacc as bacc
import concourse.mybir as mybir
import concourse.tile as tile
from concourse import bass_utils
from concourse.bass import ts

F32 = mybir.dt.float32
I32 = mybir.dt.int32

B, C, H, W = 1, 256, 64, 96
R = 4
K = 2 * R + 1          # 9
PK = K + 1             # 10 (patch side)
NQ = H * W             # 6144
NCORES = 8
QPC = NQ // NCORES     # 768
P = 128
NT = QPC // P          # 6 tiles per core
GUARD = 512            # head guard (window can start below the slot)
GUARD_TAIL = 1024      # tail guard (window can end past the last slot)


# --------------------------------------------------------------------------
# host-side preprocessing
# --------------------------------------------------------------------------

def host_preprocess(fmap1, fmap2, coords):
    """Returns (in_maps, order, NF)."""
    f1 = np.asarray(fmap1, np.float32).reshape(C, NQ)
    f2 = np.asarray(fmap2, np.float32).reshape(C, NQ)
    cx = np.asarray(coords, np.float32)[0, 0].reshape(NQ)
    cy = np.asarray(coords, np.float32)[0, 1].reshape(NQ)

    ix = np.floor(cx)
    iy = np.floor(cy)
    fx = cx - ix          # exact in fp32
    fy = cy - iy
    ixi = ix.astype(np.int64)
    iyi = iy.astype(np.int64)

    order = np.argsort(iyi, kind="stable")

    # uniform band width across cores
    w_req = 0
    for c in range(NCORES):
        qs = order[c * QPC:(c + 1) * QPC]
        w_req = max(w_req, int(iyi[qs].max() - iyi[qs].min()) + PK)
    W_ROWS = min(H, w_req)
    NF = W_ROWS * W

    in_maps = []
    for c in range(NCORES):
        qs = order[c * QPC:(c + 1) * QPC]
        miny = int(iyi[qs].min())
        r0 = int(np.clip(miny - R, 0, H - W_ROWS))

        f1s = f1[:, qs].reshape(2, P, QPC)
        # band columns reordered x-major (c*W_ROWS + r): the corr band then
        # lands in DRAM transposed per query, so a patch window spans only
        # 9*W_ROWS+10 elements instead of 9*96+10.
        f2w = f2[:, r0 * W: r0 * W + NF].reshape(C, W_ROWS, W)
        f2s = np.ascontiguousarray(f2w.transpose(0, 2, 1).reshape(2, P, NF))

        jy = iyi[qs]           # [768]
        jx = ixi[qs]
        a = np.arange(PK)      # [10]
        r_abs = jy[:, None] - R + a[None, :]            # patch row abs y
        # per-query window start (one gather offset per query)
        idx = (GUARD + (np.arange(QPC) % P) * NF
               + (jx - R) * W_ROWS + (jy - R - r0)).astype(np.int32)[:, None]

        bcol = np.arange(PK)
        mx = ((jx[:, None] - R + bcol[None, :] >= 0)
              & (jx[:, None] - R + bcol[None, :] <= W - 1))   # [768,10]
        my = (r_abs >= 0) & (r_abs <= H - 1)                  # [768,10]
        # transposed mask layout [q, b(x), a(y)]
        m2 = (mx[:, :, None] & my[:, None, :]).astype(np.float32)

        wx1 = fx[qs].astype(np.float32)
        wy1 = fy[qs].astype(np.float32)
        # inner (window-minor) axis is y -> inner mix uses wy, outer uses wx
        wts = np.stack([(1.0 - wy1), wy1,
                        (1.0 - wx1) / 16.0, wx1 / 16.0], axis=1).astype(np.float32)

        in_maps.append({
            "f1s": np.ascontiguousarray(f1s),
            "f2s": np.ascontiguousarray(f2s),
            "idx": idx,
            "m2": np.ascontiguousarray(m2.reshape(QPC, PK * PK)),
            "wts": np.ascontiguousarray(wts),
        })
    return in_maps, order, NF


def split_bf16_inputs(in_maps):
    """Replace f1s/f2s with bf16 hi/lo splits (for mm_dtype='bf16x3')."""
    import ml_dtypes
    bf16 = ml_dtypes.bfloat16
    out = []
    for m in in_maps:
        m = dict(m)
        for name in ("f1s", "f2s"):
            x = m.pop(name).astype(np.float32)
            hi = x.astype(bf16)
            lo = (x - hi.astype(np.float32)).astype(bf16)
            m[name + "h"] = hi
            m[name + "l"] = lo
        out.append(m)
    return out


def assemble_output(results, order):
    rows = np.concatenate([results[c]["out"] for c in range(NCORES)], axis=0)
    # device blend emits [dx, dy]-major, matching the reference's 81-axis
    # (delta[..., 0] is added to x and varies along the first grid axis)
    full = np.empty((K * K, NQ), np.float32)
    full[:, order] = rows.T
    return full.reshape(1, K * K, H, W)


# --------------------------------------------------------------------------
# device program
# --------------------------------------------------------------------------

def _body(tc, nc, aps, scr, NF, nchunks, mm_dtype=F32):
    idx, m2, wts, out = aps["idx"], aps["m2"], aps["wts"], aps["out"]
    bf3 = (mm_dtype == "bf16x3")
    import contextlib
    ctx = contextlib.ExitStack()
    with ctx:
        const = ctx.enter_context(tc.tile_pool(name="const", bufs=1))
        corr_pool = ctx.enter_context(tc.tile_pool(name="corr", bufs=2))
        psum_pool = ctx.enter_context(
            tc.tile_pool(name="ps", bufs=4, space="PSUM"))
        small = ctx.enter_context(tc.tile_pool(name="small", bufs=3))

        # resident inputs.  mm_list: (lhsT sbuf tile, rhs sbuf tile, k) per
        # accumulating matmul of one output chunk.
        if bf3:
            BF = mybir.dt.bfloat16
            f1bh = const.tile([P, 2 * QPC], BF)
            f1bl = const.tile([P, 2 * QPC], BF)
            f2bh0 = const.tile([P, NF], BF)
            f2bl0 = const.tile([P, NF], BF)
            f2bh1 = const.tile([P, NF], BF)
            f2bl1 = const.tile([P, NF], BF)
            for k in range(2):
                nc.sync.dma_start(f1bh[:, k * QPC:(k + 1) * QPC],
                                  aps["f1sh"][k])
                nc.sync.dma_start(f1bl[:, k * QPC:(k + 1) * QPC],
                                  aps["f1sl"][k])
            nc.sync.dma_start(f2bh0[:], aps["f2sh"][0])
            nc.sync.dma_start(f2bh1[:], aps["f2sh"][1])
            nc.sync.dma_start(f2bl0[:], aps["f2sl"][0])
            nc.sync.dma_start(f2bl1[:], aps["f2sl"][1])
            f2bh = [f2bh0, f2bh1]
            f2bl = [f2bl0, f2bl1]
            mm_list = [(f1bh, f2bh[0], 0), (f1bh, f2bh[1], 1),
                       (f1bh, f2bl[0], 0), (f1bl, f2bh[0], 0),
                       (f1bh, f2bl[1], 1), (f1bl, f2bh[1], 1)]
        else:
            f1b = const.tile([P, 2 * QPC], F32)
            nc.sync.dma_start(f1b[:, 0:QPC], aps["f1s"][0])
            nc.sync.dma_start(f1b[:, QPC:2 * QPC], aps["f1s"][1])
            f2b0 = const.tile([P, NF], F32)
            nc.sync.dma_start(f2b0[:], aps["f2s"][0])
            f2b1 = const.tile([P, NF], F32)
            nc.sync.dma_start(f2b1[:], aps["f2s"][1])
            f2b = [f2b0, f2b1]
            mm_list = [(f1b, f2b[0], 0), (f1b, f2b[1], 1)]

        idxb = const.tile([P, NT], I32)
        nc.sync.dma_start(idxb[:].rearrange("p (t a) -> p t a", a=1),
                          idx.rearrange("(t p) a -> p t a", p=P))
        m2b = const.tile([P, NT * PK * PK], F32)
        nc.sync.dma_start(m2b[:].rearrange("p (t a) -> p t a", a=PK * PK),
                          m2.rearrange("(t p) a -> p t a", p=P))
        wtsb = const.tile([P, NT * 4], F32)
        nc.sync.dma_start(wtsb[:].rearrange("p (t a) -> p t a", a=4),
                          wts.rearrange("(t p) a -> p t a", p=P))

        chunks = [(i * 512, min(512, NF - i * 512)) for i in range(nchunks)]

        # zero the scratch guard bands (a masked-out window row may read them;
        # uninitialized HBM could hold NaN and 0*NaN would poison the blend)
        zt = const.tile([1, GUARD_TAIL], F32)
        nc.vector.memset(zt[:], 0.0)
        for t in range(NT):
            g = scr[t].ap()[0:GUARD].rearrange("(p f) -> p f", p=1)
            nc.sync.dma_start(g, zt[:, 0:GUARD])
            g = scr[t].ap()[GUARD + P * NF:GUARD + P * NF + GUARD_TAIL] \
                .rearrange("(p f) -> p f", p=1)
            nc.sync.dma_start(g, zt[:])

        for t in range(NT):
            corr_sb = corr_pool.tile([P, NF], F32)
            for ci, (c0, cw) in enumerate(chunks):
                ps = psum_pool.tile([P, 512], F32, space="PSUM", tag="ps")
                for mi, (f1t, f2t, k) in enumerate(mm_list):
                    lhsT = f1t[:, k * QPC + t * P: k * QPC + (t + 1) * P]
                    rhs = f2t[:, c0:c0 + cw]
                    if not bf3 and mm_dtype != F32:
                        lhsT = lhsT.bitcast(mm_dtype)
                        rhs = rhs.bitcast(mm_dtype)
                    nc.tensor.matmul(
                        ps[:, :cw], lhsT=lhsT, rhs=rhs,
                        start=(mi == 0), stop=(mi == len(mm_list) - 1))
                # alternate PSUM->SBUF copies across ACT and DVE
                if ci % 2 == 0:
                    nc.scalar.copy(corr_sb[:, c0:c0 + cw], ps[:, :cw])
                else:
                    nc.vector.tensor_copy(corr_sb[:, c0:c0 + cw], ps[:, :cw])

            dst = scr[t].ap()[GUARD:GUARD + P * NF] \
                .rearrange("(p f) -> p f", p=P)
            nc.sync.dma_start(dst, corr_sb[:])

            wrows = NF // W
            win = (PK - 1) * wrows + PK
            pt = small.tile([P, PK * wrows], F32, tag="pt")
            src = scr[t].ap().rearrange("(n o) -> n o", o=1)
            nc.gpsimd.indirect_dma_start(
                out=pt[:, 0:win], out_offset=None, in_=src,
                in_offset=bass.IndirectOffsetOnAxis(
                    ap=idxb[:, t:t + 1], axis=0))
            # patch view: x-strips at stride wrows inside the gathered window
            ptv = pt[:].rearrange("p (b r) -> p b r", r=wrows)[:, :, 0:PK]

            pm = small.tile([P, PK * PK], F32, tag="pm")
            nc.vector.tensor_tensor(
                pm[:].rearrange("p (a b) -> p a b", b=PK), ptv,
                m2b[:, ts(t, PK * PK)].rearrange("p (a b) -> p a b", b=PK),
                op=mybir.AluOpType.mult)
            pm3 = pm[:].rearrange("p (a b) -> p a b", b=PK)

            t1 = small.tile([P, PK * K], F32, tag="t1")
            t13 = t1[:].rearrange("p (a b) -> p a b", b=K)
            nc.vector.tensor_scalar_mul(
                t13, pm3[:, :, 1:PK], wtsb[:, 4 * t + 1: 4 * t + 2])
            cm = small.tile([P, PK * K], F32, tag="cm")
            cm3 = cm[:].rearrange("p (a b) -> p a b", b=K)
            nc.vector.scalar_tensor_tensor(
                cm3, pm3[:, :, 0:K], wtsb[:, 4 * t: 4 * t + 1], t13,
                op0=mybir.AluOpType.mult, op1=mybir.AluOpType.add)

            t2 = small.tile([P, K * K], F32, tag="t2")
            t23 = t2[:].rearrange("p (a b) -> p a b", b=K)
            nc.vector.tensor_scalar_mul(
                t23, cm3[:, 1:PK, :], wtsb[:, 4 * t + 3: 4 * t + 4])
            ot = small.tile([P, K * K], F32, tag="ot")
            ot3 = ot[:].rearrange("p (a b) -> p a b", b=K)
            nc.vector.scalar_tensor_tensor(
                ot3, cm3[:, 0:K, :], wtsb[:, 4 * t + 2: 4 * t + 3], t23,
                op0=mybir.AluOpType.mult, op1=mybir.AluOpType.add)

            nc.sync.dma_start(out[ts(t, P), :], ot[:])


def build_program(NF, rep=1, mm_dtype=F32):
    """rep>1 wraps the body in a For_i loop (for wall-clock timing)."""
    nchunks = (NF + 511) // 512
    nc = bacc.Bacc("TRN2", target_bir_lowering=False, debug=False,
                   num_devices=NCORES)
    aps = {}
    if mm_dtype == "bf16x3":
        BF = mybir.dt.bfloat16
        for nm in ("f1sh", "f1sl"):
            aps[nm] = nc.dram_tensor(nm, [2, P, QPC], BF,
                                     kind="ExternalInput").ap()
        for nm in ("f2sh", "f2sl"):
            aps[nm] = nc.dram_tensor(nm, [2, P, NF], BF,
                                     kind="ExternalInput").ap()
    else:
        aps["f1s"] = nc.dram_tensor("f1s", [2, P, QPC], F32,
                                    kind="ExternalInput").ap()
        aps["f2s"] = nc.dram_tensor("f2s", [2, P, NF], F32,
                                    kind="ExternalInput").ap()
    aps["idx"] = nc.dram_tensor("idx", [QPC, 1], I32,
                                kind="ExternalInput").ap()
    aps["m2"] = nc.dram_tensor("m2", [QPC, PK * PK], F32,
                               kind="ExternalInput").ap()
    aps["wts"] = nc.dram_tensor("wts", [QPC, 4], F32,
                                kind="ExternalInput").ap()
    aps["out"] = nc.dram_tensor("out", [QPC, K * K], F32,
                                kind="ExternalOutput").ap()
    scr = [nc.dram_tensor(f"scr{t}", [GUARD + P * NF + GUARD_TAIL], F32)
           for t in range(NT)]

    with tile.TileContext(nc) as tc:
        if rep == 1:
            _body(tc, nc, aps, scr, NF, nchunks, mm_dtype)
        else:
            with tc.For_i(0, rep):
                _body(tc, nc, aps, scr, NF, nchunks, mm_dtype)
    nc.compile()
    return nc


_PROGRAMS = {}


def kernel(fmap1, fmap2, coords, radius):
    assert int(radius) == R, f"kernel hardcodes radius=4, got {radius}"
    in_maps, order, NF = host_preprocess(fmap1, fmap2, coords)
    in_maps = split_bf16_inputs(in_maps)
    nc = _PROGRAMS.get(NF)
    if nc is None:
        nc = _PROGRAMS[NF] = build_program(NF, mm_dtype="bf16x3")
    last_err = None
    for _ in range(3):  # the remote compile hook occasionally flakes
        try:
            res = bass_utils.run_bass_kernel_spmd(
                nc, in_maps, core_ids=list(range(NCORES)))
            return assemble_output(res.results, order)
        except Exception as e:  # noqa: BLE001
            last_err = e
    raise last_err



# revision 6
# speedup vs baseline: 1.3310x; 1.3310x over previous
"""Trainium2 Bass kernel for nn_CorrBlockSingleScale (RAFT single-scale
correlation lookup), distributed over 8 NeuronCores.

  fmap1, fmap2: [1, 256, 64, 96] f32;  coords: [1, 2, 64, 96] f32; radius=4
  corr = einsum('bcm,bcn->bmn', f1, f2) / 16        -> [6144, 64, 96]
  out[q, i, j] = bilinear(corr[q], (cx_q + d_i, cy_q + d_j)),  d in -4..4
  output [1, 81, 64, 96] f32.

Structure exploited: the 9x9 sample offsets are integers, so all 81 samples
of a query share one fractional pair (fx, fy) -- the output is a separable
2x2-tap blend of a 10x10 patch of corr[q] anchored at
(floor(cx)-4, floor(cy)-4).

Distribution (no collectives): queries are sorted by floor(cy) on the host;
each core takes 768 contiguous sorted queries and therefore only needs a
narrow y-band (~19 of 64 rows) of the correlation target plane.  Within a
core the 768 queries are further sorted by floor(cx), so each 128-query
tile only touches a ~26-32 x-column slice of the band.  The rhs slice
boundaries are baked as the UNION of the per-core tile x-ranges (the 8
cores run one shared SPMD program), which costs only a few extra columns
since per-core x-quantiles are tight.

Per core:
  1. one packed DMA loads f1 (bf16) + the core's x-major f2 band (bf16);
     one packed DMA loads idx/masks/weights.
  2. per tile: 2 accumulating bf16 matmuls per <=512-col chunk compute the
     tile's corr slice; PSUM->SBUF copies (alternating ACT/DVE) downconvert
     to bf16 into a per-pair staging buffer.
  3. per pair of tiles: one DMA writes the staged corr to a DRAM scratch
     slot; one indirect DMA gathers each query's contiguous 181-element
     window (the 10x10 patch spans 9*19+10 elements in the x-major layout).
  4. blend: mask multiply (DVE), y-mix (ACT mul + DVE scalar_tensor_tensor),
     x-mix (ACT mul + DVE stt) with host-folded bilinear weights; results
     accumulate in SBUF and are written out in one DMA per core.
Host post-pass inverse-permutes to the reference layout.
"""

import numpy as np

import concourse.bass as bass
import concourse.bacc as bacc
import concourse.mybir as mybir
import concourse.tile as tile
from concourse import bass_utils

F32 = mybir.dt.float32
BF16 = mybir.dt.bfloat16
I32 = mybir.dt.int32

B, C, H, W = 1, 256, 64, 96
R = 4
K = 2 * R + 1          # 9
PK = K + 1             # 10 (patch side)
NQ = H * W             # 6144
NCORES = 8
QPC = NQ // NCORES     # 768
P = 128
NT = QPC // P          # 6 tiles per core
NPAIR = NT // 2        # 3 scratch/gather pairs
GS = 96                # scratch head guard (elements)
GT = 192               # scratch tail guard
SROW = 56              # small-pack row: 6 idx-cols handled separately; see below
WIN = K * 0 + 0        # computed per-params


# --------------------------------------------------------------------------
# host-side preprocessing
# --------------------------------------------------------------------------

def host_preprocess(fmap1, fmap2, coords):
    """Returns (in_maps, order, params).

    params = (W_ROWS, xlo_u (tuple of NT), nx_u (tuple of NT)) -- the baked
    per-tile rhs slice bounds, uniform across cores.
    """
    import ml_dtypes
    bf16 = ml_dtypes.bfloat16

    f1 = np.asarray(fmap1, np.float32).reshape(C, NQ)
    f2 = np.asarray(fmap2, np.float32).reshape(C, NQ)
    cx = np.asarray(coords, np.float32)[0, 0].reshape(NQ)
    cy = np.asarray(coords, np.float32)[0, 1].reshape(NQ)

    ix = np.floor(cx)
    iy = np.floor(cy)
    fx = (cx - ix).astype(np.float32)
    fy = (cy - iy).astype(np.float32)
    ixi = ix.astype(np.int64)
    iyi = iy.astype(np.int64)

    order0 = np.argsort(iyi, kind="stable")
    order = np.empty_like(order0)
    for c in range(NCORES):
        blk = order0[c * QPC:(c + 1) * QPC]
        order[c * QPC:(c + 1) * QPC] = blk[np.argsort(ixi[blk], kind="stable")]

    # uniform band height across cores
    w_req = 0
    for c in range(NCORES):
        qs = order[c * QPC:(c + 1) * QPC]
        w_req = max(w_req, int(iyi[qs].max() - iyi[qs].min()) + PK)
    W_ROWS = min(H, w_req)

    # union per-tile x-slices across cores, clipped to the image (taps at
    # x<0 / x>=W read guards or neighbor regions and are masked out)
    xlo_u = [10 ** 9] * NT
    xhi_u = [-10 ** 9] * NT
    for c in range(NCORES):
        qs = order[c * QPC:(c + 1) * QPC]
        for t in range(NT):
            jx = ixi[qs[t * P:(t + 1) * P]]
            xlo_u[t] = min(xlo_u[t], max(0, int(jx.min()) - R))
            xhi_u[t] = max(xhi_u[t], min(W, int(jx.max()) + R + 2))
    nx_u = [xhi_u[t] - xlo_u[t] for t in range(NT)]
    params = (W_ROWS, tuple(xlo_u), tuple(nx_u))

    CW = [nx_u[t] * W_ROWS for t in range(NT)]
    NF = W_ROWS * W

    in_maps = []
    for c in range(NCORES):
        qs = order[c * QPC:(c + 1) * QPC]
        miny = int(iyi[qs].min())
        r0 = int(np.clip(miny - R, 0, H - W_ROWS))

        f1r = f1[:, qs].reshape(2, P, QPC)
        # band columns x-major (x*W_ROWS + r): a query's 10x10 patch then
        # spans 9*W_ROWS+10 contiguous-ish elements (one gather per query)
        f2w = f2[:, r0 * W: r0 * W + NF].reshape(C, W_ROWS, W)
        f2s = np.ascontiguousarray(
            f2w.transpose(0, 2, 1).reshape(2, P, NF))

        fm = np.empty((P, 2 * QPC + 2 * NF), np.float32)
        fm[:, 0:QPC] = f1r[0]
        fm[:, QPC:2 * QPC] = f1r[1]
        fm[:, 2 * QPC:2 * QPC + NF] = f2s[0]
        fm[:, 2 * QPC + NF:] = f2s[1]

        jy = iyi[qs]
        jx = ixi[qs]

        # per-query gather offsets into the per-pair scratch slots
        idx = np.empty(QPC, np.int32)
        for t in range(NT):
            j, i = divmod(t, 2)
            cw_pair = CW[2 * j] + CW[2 * j + 1]
            sl = slice(t * P, (t + 1) * P)
            idx[sl] = (GS + np.arange(P) * cw_pair
                       + (CW[2 * j] if i == 1 else 0)
                       + (jx[sl] - R - xlo_u[t]) * W_ROWS
                       + (jy[sl] - R - r0)).astype(np.int32)

        a = np.arange(PK)
        r_abs = jy[:, None] - R + a[None, :]
        mx = ((jx[:, None] - R + a[None, :] >= 0)
              & (jx[:, None] - R + a[None, :] <= W - 1))     # [768,10] (x)
        my = (r_abs >= 0) & (r_abs <= H - 1)                 # [768,10] (y)
        m2 = (mx[:, :, None] & my[:, None, :]).astype(bf16)  # [q, b(x), a(y)]

        wx1 = fx[qs]
        wy1 = fy[qs]
        wts = np.stack([(1.0 - wy1), wy1,
                        (1.0 - wx1) / 16.0, wx1 / 16.0],
                       axis=1).astype(np.float32)

        # small-pack layout per partition p (f32 elems):
        #   cols [0, 6)          idx (i32 bits) for tiles 0..5
        #   cols 6+t*55 + [0,50) m2 bf16 bits (100 bf16 = 50 f32)
        #   cols 6+t*55+[50,54)  wts
        #   col  6+t*55+54       pad
        small = np.zeros((P, 6 + NT * 55), np.float32)
        sm_i32 = small.view(np.int32)
        sm_bf = small.view(bf16)
        for t in range(NT):
            sl = slice(t * P, (t + 1) * P)
            sm_i32[:, t] = idx[sl]
            base = 6 + t * 55
            sm_bf[:, 2 * base:2 * base + 100] = \
                m2[sl].reshape(P, PK * PK)
            small[:, base + 50:base + 54] = wts[sl]

        in_maps.append({
            "fm": fm.astype(bf16),
            "small": small,
        })
    return in_maps, order, params


def assemble_output(results, order):
    # device emits [128, 6*81] bf16 per core; row (t*128+p) of the core's
    # query block is buf[p, t*81:(t+1)*81]; 81-axis is [dx, dy]-major,
    # matching the reference's delta layout.
    rows = np.empty((NQ, K * K), np.float32)
    for c in range(NCORES):
        buf = np.asarray(results[c]["out"], np.float32)
        rows[c * QPC:(c + 1) * QPC] = \
            buf.reshape(P, NT, K * K).transpose(1, 0, 2).reshape(QPC, K * K)
    full = np.empty((K * K, NQ), np.float32)
    full[:, order] = rows.T
    return full.reshape(1, K * K, H, W)


# --------------------------------------------------------------------------
# device program
# --------------------------------------------------------------------------

def _body(tc, nc, aps, scr, params):
    W_ROWS, xlo_u, nx_u = params
    NF = W_ROWS * W
    CW = [nx_u[t] * W_ROWS for t in range(NT)]
    win = (PK - 1) * W_ROWS + PK
    wrow = PK * W_ROWS           # gather dst stride per tile (>= win)

    import contextlib
    ctx = contextlib.ExitStack()
    with ctx:
        const = ctx.enter_context(tc.tile_pool(name="const", bufs=1))
        corr_pool = ctx.enter_context(tc.tile_pool(name="corr", bufs=2))
        psum_pool = ctx.enter_context(
            tc.tile_pool(name="ps", bufs=6, space="PSUM"))
        small = ctx.enter_context(tc.tile_pool(name="small", bufs=3))

        fm = const.tile([P, 2 * QPC + 2 * NF], BF16)
        nc.sync.dma_start(fm[:], aps["fm"])
        smt = const.tile([P, 6 + NT * 55], F32)
        nc.sync.dma_start(smt[:], aps["small"])
        outb = const.tile([P, NT * K * K], BF16)

        copy_ctr = [0]

        def mm_tile(t, corrb, off):
            """matmuls + PSUM->SBUF(bf16) copies for tile t."""
            cw_t = CW[t]
            base2 = 2 * QPC
            # chunk split (<=512 cols per PSUM bank)
            nchunk = (cw_t + 511) // 512
            bnds = []
            pos = 0
            nxs = nx_u[t] // nchunk
            for ci in range(nchunk):
                nxc = nxs if ci < nchunk - 1 else nx_u[t] - nxs * (nchunk - 1)
                bnds.append((pos, nxc * W_ROWS))
                pos += nxc * W_ROWS
            pss = [psum_pool.tile([P, 512], F32, space="PSUM", tag="ps",
                                  name=f"ps_{t}_{ci}")
                   for ci in range(nchunk)]
            for k in range(2):
                lhsT = fm[:, k * QPC + t * P: k * QPC + (t + 1) * P]
                for ci, (c0, cwc) in enumerate(bnds):
                    rhs = fm[:, base2 + k * NF + xlo_u[t] * W_ROWS + c0:
                             base2 + k * NF + xlo_u[t] * W_ROWS + c0 + cwc]
                    nc.tensor.matmul(pss[ci][:, :cwc], lhsT=lhsT, rhs=rhs,
                                     start=(k == 0), stop=(k == 1))
            for ci, (c0, cwc) in enumerate(bnds):
                dst = corrb[:, off + c0: off + c0 + cwc]
                if copy_ctr[0] % 2 == 0:
                    nc.scalar.copy(dst, pss[ci][:, :cwc])
                else:
                    nc.vector.tensor_copy(dst, pss[ci][:, :cwc])
                copy_ctr[0] += 1

        def blend_tile(t, pt, ipt):
            base = 6 + t * 55
            ptv = pt[:, ipt * wrow: ipt * wrow + wrow] \
                .rearrange("p (b r) -> p b r", r=W_ROWS)[:, :, 0:PK]
            m2v = smt[:, base: base + 50].bitcast(BF16) \
                .rearrange("p (a b) -> p a b", b=PK)
            w0 = smt[:, base + 50: base + 51]
            w1 = smt[:, base + 51: base + 52]
            w2 = smt[:, base + 52: base + 53]
            w3 = smt[:, base + 53: base + 54]

            pm = small.tile([P, PK * PK], F32, tag="pm")
            pm3 = pm[:].rearrange("p (a b) -> p a b", b=PK)
            nc.vector.tensor_tensor(pm3, ptv, m2v, op=mybir.AluOpType.mult)

            t1 = small.tile([P, PK * K], F32, tag="t1")
            t13 = t1[:].rearrange("p (a b) -> p a b", b=K)
            nc.scalar.mul(t13, pm3[:, :, 1:PK], w1)
            cm = small.tile([P, PK * K], F32, tag="cm")
            cm3 = cm[:].rearrange("p (a b) -> p a b", b=K)
            nc.vector.scalar_tensor_tensor(
                cm3, pm3[:, :, 0:K], w0, t13,
                op0=mybir.AluOpType.mult, op1=mybir.AluOpType.add)

            t2 = small.tile([P, K * K], F32, tag="t2")
            t23 = t2[:].rearrange("p (a b) -> p a b", b=K)
            nc.scalar.mul(t23, cm3[:, 1:PK, :], w3)
            ot3 = outb[:, t * K * K:(t + 1) * K * K] \
                .rearrange("p (a b) -> p a b", b=K)
            nc.vector.scalar_tensor_tensor(
                ot3, cm3[:, 0:K, :], w2, t23,
                op0=mybir.AluOpType.mult, op1=mybir.AluOpType.add)

        for j in range(NPAIR):
            ta, tb = 2 * j, 2 * j + 1
            cw_pair = CW[ta] + CW[tb]
            corrb = corr_pool.tile([P, cw_pair], BF16, tag="corr")
            mm_tile(ta, corrb, 0)
            mm_tile(tb, corrb, CW[ta])

            dst = scr[j].ap()[GS:GS + P * cw_pair] \
                .rearrange("(p f) -> p f", p=P)
            nc.sync.dma_start(dst, corrb[:])

            # NOTE: the SWDGE descgen consumes ONE offset per partition and
            # scales multi-run dst offsets by the run stride, so each tile
            # needs its own gather (pair-batched gathers mis-read on HW).
            src = scr[j].ap().rearrange("(n o) -> n o", o=1)
            pts = []
            for i, t in ((0, ta), (1, tb)):
                pt = small.tile([P, wrow], BF16, tag=f"pt{i}",
                                name=f"pt_{j}_{i}")
                nc.gpsimd.indirect_dma_start(
                    out=pt[:], out_offset=None, in_=src,
                    in_offset=bass.IndirectOffsetOnAxis(
                        ap=smt[:, t:t + 1].bitcast(I32), axis=0))
                pts.append(pt)

            blend_tile(ta, pts[0], 0)
            blend_tile(tb, pts[1], 0)

        nc.sync.dma_start(aps["out"], outb[:])


def build_program(params, rep=1):
    """rep>1 wraps the body in a For_i loop (for wall-clock timing)."""
    W_ROWS, xlo_u, nx_u = params
    NF = W_ROWS * W
    CW = [nx_u[t] * W_ROWS for t in range(NT)]

    nc = bacc.Bacc("TRN2", target_bir_lowering=False, debug=False,
                   num_devices=NCORES)
    aps = {
        "fm": nc.dram_tensor("fm", [P, 2 * QPC + 2 * NF], BF16,
                             kind="ExternalInput").ap(),
        "small": nc.dram_tensor("small", [P, 6 + NT * 55], F32,
                                kind="ExternalInput").ap(),
        "out": nc.dram_tensor("out", [P, NT * K * K], BF16,
                              kind="ExternalOutput").ap(),
    }
    scr = [nc.dram_tensor(
        f"scr{j}", [GS + P * (CW[2 * j] + CW[2 * j + 1]) + GT], BF16)
        for j in range(NPAIR)]

    with tile.TileContext(nc) as tc:
        # preamble (outside the timed loop): zero the scratch guard bands.
        # a masked-out window may read them; uninitialized HBM could hold
        # NaN and 0*NaN would poison the blend.
        with tc.tile_pool(name="zz", bufs=1) as zp:
            zt = zp.tile([1, max(GS, GT)], BF16)
            nc.vector.memset(zt[:], 0.0)
            for j in range(NPAIR):
                g = scr[j].ap()[0:GS].rearrange("(p f) -> p f", p=1)
                nc.sync.dma_start(g, zt[:, 0:GS])
                n = GS + P * (CW[2 * j] + CW[2 * j + 1]) + GT
                g = scr[j].ap()[n - GT:n].rearrange("(p f) -> p f", p=1)
                nc.sync.dma_start(g, zt[:, 0:GT])
            if rep == 1:
                _body(tc, nc, aps, scr, params)
            else:
                with tc.For_i(0, rep):
                    _body(tc, nc, aps, scr, params)
    nc.compile()
    return nc


_PROGRAMS = {}


def kernel(fmap1, fmap2, coords, radius):
    assert int(radius) == R, f"kernel hardcodes radius=4, got {radius}"
    in_maps, order, params = host_preprocess(fmap1, fmap2, coords)
    nc = _PROGRAMS.get(params)
    if nc is None:
        nc = _PROGRAMS[params] = build_program(params)
    last_err = None
    for _ in range(3):  # the remote compile hook occasionally flakes
        try:
            res = bass_utils.run_bass_kernel_spmd(
                nc, in_maps, core_ids=list(range(NCORES)))
            return assemble_output(res.results, order)
        except Exception as e:  # noqa: BLE001
            last_err = e
    raise last_err
